# revision 3
# baseline (speedup 1.0000x reference)
"""Trainium2 Bass kernel for nn_Net_89094801588965 (moe_routing).

Data-parallel over batch on 8 NeuronCores. Per-core layout puts features on
SBUF partitions and batch on the free dim, so every layer's output is directly
the next layer's moving operand (no transposes on device).

Math (identical to the reference):
  h  = relu(x @ fc1_w + b) -> relu(@fc2_w+b) -> relu(@fc3_w+b)   [B,256]
  p  = relu(x @ priv_w[task_id] + priv_b[task_id])               [B,256]
  xc = [p, h]                                                    [B,512]
  per-task heads t=0..9: a3[t] = (relu(relu(xc@h1w[t]+b)@h2w[t]+b))@h3w[t]+b
  out[b] = a3[tt[b]][b]

Device-side restructuring:
  - fc1 and the private layer share the input x -> fused into one [784,656]
    matmul (cols 0..255 = private, 256..655 = fc1).
  - head layer 1: all tasks packed as [512, 320] (task t at cols 32t..32t+27,
    zero padded) -> [320, N] activations.
  - head layer 2: block-diagonal [320, 320], 128-aligned diagonal blocks ->
    3 matmuls (tasks 0-3, 4-7, 8-9).
  - head layer 3: block-structured [320, 100] -> a3 for all tasks as [100, N].
  - selection: one-hot mask over tasks, expanded to 100 rows on the host,
    applied as (a3 + bias) * mask, then reduced with a constant [100, 10]
    summing matrix on the PE.  (relu/bias of wrong tasks is killed by the
    mask, so masking once at the end is exact.)
All matmuls run as float32r (full PE rate, ~1e-4 relative error).
"""

import sys

sys.path.insert(0, "/opt/trn_rl_repo")

import numpy as np

import concourse.bass as bass
import concourse.mybir as mybir
import concourse.tile as tile
from concourse import bacc
from concourse.bass_utils import run_bass_kernel_spmd

F32 = mybir.dt.float32
F32R = mybir.dt.float32r
RELU = mybir.ActivationFunctionType.Relu

B = 65536
D = 784
HID = 400
LAT = 256
T = 10
NCLS = 10
NCORES = 8
R = B // NCORES          # rows per core
CH = 512                 # batch columns per chunk
NCH = R // CH            # chunks per core

M1 = LAT + HID           # 656 fused L1 output (private | fc1)
HP = 32                  # per-task padded head width
HT = T * HP              # 320
A3 = T * NCLS            # 100

_cache = {}


def _ceil_tiles(n):
    full, rem = divmod(n, 128)
    sizes = [128] * full
    if rem:
        sizes.append(rem)
    return sizes


L1_K = _ceil_tiles(D)            # [128]*6 + [16]
L1_M = _ceil_tiles(M1)           # [128]*5 + [16]
L2_K = _ceil_tiles(HID)          # [128]*3 + [16]
L2_M = _ceil_tiles(HID)
L3_M = _ceil_tiles(LAT)          # [128, 128]
H1_K = _ceil_tiles(2 * LAT)      # [128]*4
H1_M = _ceil_tiles(HT)           # [128, 128, 64]
H3_K = _ceil_tiles(HT)           # [128, 128, 64]


def _build_program():
    nc = bacc.Bacc("TRN2", target_bir_lowering=False, debug=False,
                   num_devices=NCORES)

    xT_d = nc.dram_tensor("xT", [D, R], F32R, kind="ExternalInput")
    mk_d = nc.dram_tensor("mask", [A3, R], F32, kind="ExternalInput")
    w1_d = nc.dram_tensor("w1", [D, M1], F32R, kind="ExternalInput")
    w2_d = nc.dram_tensor("w2", [HID, HID], F32R, kind="ExternalInput")
    w3_d = nc.dram_tensor("w3", [HID, LAT], F32R, kind="ExternalInput")
    wh1_d = nc.dram_tensor("wh1", [2 * LAT, HT], F32R, kind="ExternalInput")
    wh2_d = nc.dram_tensor("wh2", [HT, HT], F32R, kind="ExternalInput")
    wh3_d = nc.dram_tensor("wh3", [HT, A3], F32R, kind="ExternalInput")
    s_d = nc.dram_tensor("ssum", [A3, NCLS], F32R, kind="ExternalInput")
    b1_d = nc.dram_tensor("b1", [128, len(L1_M)], F32, kind="ExternalInput")
    b2_d = nc.dram_tensor("b2", [128, len(L2_M)], F32, kind="ExternalInput")
    b3_d = nc.dram_tensor("b3", [128, len(L3_M)], F32, kind="ExternalInput")
    bh1_d = nc.dram_tensor("bh1", [128, len(H1_M)], F32, kind="ExternalInput")
    bh2_d = nc.dram_tensor("bh2", [128, len(H1_M)], F32, kind="ExternalInput")
    bh3_d = nc.dram_tensor("bh3", [A3, 1], F32, kind="ExternalInput")
    out_d = nc.dram_tensor("out", [NCLS, R], F32, kind="ExternalOutput")

    with tile.TileContext(nc) as tc:
        with (
            tc.tile_pool(name="wp", bufs=1) as wp,
            tc.tile_pool(name="xp", bufs=14) as xp,
            tc.tile_pool(name="mp", bufs=2) as mpool,
            tc.tile_pool(name="ap", bufs=2) as ap,
            tc.tile_pool(name="op", bufs=2) as op,
            tc.tile_pool(name="ps", bufs=8, space="PSUM") as ps,
        ):
            # ---- resident weights -------------------------------------
            def load_w(dram, ksizes, ncols, tag):
                tiles = []
                r0 = 0
                for i, kp in enumerate(ksizes):
                    t = wp.tile([kp, ncols], F32R, tag=f"{tag}{i}")
                    nc.sync.dma_start(t[:], dram[r0:r0 + kp, :])
                    tiles.append(t)
                    r0 += kp
                return tiles

            w1 = load_w(w1_d, L1_K, M1, "w1")
            w2 = load_w(w2_d, L2_K, HID, "w2")
            w3 = load_w(w3_d, L2_K, LAT, "w3")
            wh1 = load_w(wh1_d, H1_K, HT, "wh1")
            wh3 = load_w(wh3_d, H3_K, A3, "wh3")
            # block-diagonal head-2: only the diagonal 128-blocks
            wh2 = []
            r0 = 0
            for i, kp in enumerate(H3_K):
                t = wp.tile([kp, kp], F32R, tag=f"wh2{i}")
                nc.sync.dma_start(t[:], wh2_d[r0:r0 + kp, r0:r0 + kp])
                wh2.append(t)
                r0 += kp
            s_sb = wp.tile([A3, NCLS], F32R, tag="ssum")
            nc.sync.dma_start(s_sb[:], s_d[:])

            def load_b(dram, ncols, tag):
                t = wp.tile([128, ncols], F32, tag=tag)
                nc.sync.dma_start(t[:], dram[:])
                return t

            b1 = load_b(b1_d, len(L1_M), "b1")
            b2 = load_b(b2_d, len(L2_M), "b2")
            b3 = load_b(b3_d, len(L3_M), "b3")
            bh1 = load_b(bh1_d, len(H1_M), "bh1")
            bh2 = load_b(bh2_d, len(H1_M), "bh2")
            bh3 = wp.tile([A3, 1], F32, tag="bh3")
            nc.sync.dma_start(bh3[:], bh3_d[:])

            # ---- per-chunk pipeline -----------------------------------
            def mm_layer(ktiles, wtiles, msizes, psum_tag_prefix, ci):
                """K-accumulated matmuls for one dense layer; returns psum
                tiles (one per m-tile)."""
                psums = []
                c0 = 0
                for mi, mp_ in enumerate(msizes):
                    pt = ps.tile([mp_, CH], F32, tag="ps")
                    nk = len(ktiles)
                    for ki in range(nk):
                        nc.tensor.matmul(
                            pt[:], wtiles[ki][:, c0:c0 + mp_],
                            ktiles[ki][:],
                            start=(ki == 0), stop=(ki == nk - 1),
                        )
                    psums.append(pt)
                    c0 += mp_
                return psums

            def act_relu(psums, bias, msizes, tag, ci):
                outs = []
                for mi, mp_ in enumerate(msizes):
                    t = ap.tile([mp_, CH], F32R, tag=f"{tag}{mi}")
                    nc.scalar.activation(t[:], psums[mi][:], RELU,
                                         bias=bias[:mp_, mi:mi + 1], scale=1.0)
                    outs.append(t)
                return outs

            deferred_tail = []

            for ci in range(NCH):
                cs = ci * CH
                # loads
                xk = []
                r0 = 0
                for ki, kp in enumerate(L1_K):
                    t = xp.tile([kp, CH], F32R, tag="x")
                    nc.sync.dma_start(t[:], xT_d[r0:r0 + kp, cs:cs + CH])
                    xk.append(t)
                    r0 += kp
                mk = mpool.tile([A3, CH], F32, tag="mask")
                nc.sync.dma_start(mk[:], mk_d[:, cs:cs + CH])

                # L1 fused (private | fc1)
                ps1 = mm_layer(xk, w1, L1_M, "l1", ci)
                a_l1 = act_relu(ps1, b1, L1_M, "l1o", ci)
                # split: cols 0..255 private (x2 low), 256..655 fc1 hidden
                x2 = [a_l1[0], a_l1[1]]          # p part, [128],[128]
                h1t = [a_l1[2], a_l1[3], a_l1[4], a_l1[5]]  # 128,128,128,16

                # run previous chunk's tail now: its inputs are long since
                # ready, and the PE stays busy with this chunk's L1 above.
                while deferred_tail:
                    deferred_tail.pop(0)()

                # L2
                ps2 = mm_layer(h1t, w2, L2_M, "l2", ci)
                h2t = act_relu(ps2, b2, L2_M, "l2o", ci)
                # L3
                ps3 = mm_layer(h2t, w3, L3_M, "l3", ci)
                x2 += act_relu(ps3, b3, L3_M, "l3o", ci)   # h part

                # head layer 1: [512] -> [320]
                ph1 = mm_layer(x2, wh1, H1_M, "h1", ci)
                a1 = act_relu(ph1, bh1, H1_M, "a1", ci)
                # head layer 2: block diagonal, 3 independent matmuls
                ph2 = []
                for i, kp in enumerate(H3_K):
                    pt = ps.tile([kp, CH], F32, tag="ps")
                    nc.tensor.matmul(pt[:], wh2[i][:], a1[i][:],
                                     start=True, stop=True)
                    ph2.append(pt)
                a2 = act_relu(ph2, bh2, H1_M, "a2", ci)
                # head layer 3: [320] -> [100] (all tasks' logits)
                pt3 = ps.tile([A3, CH], F32, tag="ps")
                for i, kp in enumerate(H3_K):
                    nc.tensor.matmul(pt3[:], wh3[i][:], a2[i][:],
                                     start=(i == 0), stop=(i == len(H3_K) - 1))

                def tail(pt3=pt3, mk=mk, cs=cs):
                    # (a3 + bias) * onehot-mask, then sum tasks via S
                    t2 = ap.tile([A3, CH], F32R, tag="t2")
                    nc.vector.scalar_tensor_tensor(
                        t2[:], pt3[:], bh3[:, 0:1], mk[:],
                        op0=mybir.AluOpType.add, op1=mybir.AluOpType.mult,
                    )
                    po = ps.tile([NCLS, CH], F32, tag="ps")
                    nc.tensor.matmul(po[:], s_sb[:], t2[:],
                                     start=True, stop=True)
                    ot = op.tile([NCLS, CH], F32, tag="o")
                    nc.scalar.copy(ot[:], po[:])
                    nc.sync.dma_start(out_d[:, cs:cs + CH], ot[:])

                deferred_tail.append(tail)

            while deferred_tail:
                deferred_tail.pop(0)()

    nc.compile()
    return nc


def _prepare_inputs(x_s, tt, task_id,
                    fc1_w, fc1_b, fc2_w, fc2_b, fc3_w, fc3_b,
                    priv_w, priv_b, h1_w, h1_b, h2_w, h2_b, h3_w, h3_b):
    f = np.float32
    task_id = int(task_id)

    x2d = np.asarray(x_s, f).reshape(B, D)
    tt = np.asarray(tt).astype(np.int64).reshape(B)

    w1 = np.concatenate([np.asarray(priv_w[task_id], f),
                         np.asarray(fc1_w, f)], axis=1)          # [784, 656]
    b1v = np.concatenate([np.asarray(priv_b[task_id], f),
                          np.asarray(fc1_b, f)])                  # [656]
    w2 = np.ascontiguousarray(np.asarray(fc2_w, f))
    w3 = np.ascontiguousarray(np.asarray(fc3_w, f))
    b2v = np.asarray(fc2_b, f)
    b3v = np.asarray(fc3_b, f)

    wh1 = np.zeros((2 * LAT, HT), f)
    bh1v = np.zeros(HT, f)
    wh2 = np.zeros((HT, HT), f)
    bh2v = np.zeros(HT, f)
    wh3 = np.zeros((HT, A3), f)
    bh3v = np.zeros(A3, f)
    for t in range(T):
        c = HP * t
        wh1[:, c:c + 28] = np.asarray(h1_w[t], f)
        bh1v[c:c + 28] = np.asarray(h1_b[t], f)
        wh2[c:c + 28, c:c + 28] = np.asarray(h2_w[t], f)
        bh2v[c:c + 28] = np.asarray(h2_b[t], f)
        wh3[c:c + 28, NCLS * t:NCLS * (t + 1)] = np.asarray(h3_w[t], f)
        bh3v[NCLS * t:NCLS * (t + 1)] = np.asarray(h3_b[t], f)
    ssum = np.zeros((A3, NCLS), f)
    for t in range(T):
        ssum[NCLS * t:NCLS * (t + 1), :] = np.eye(NCLS, dtype=f)

    def col_bias(v, msizes):
        out = np.zeros((128, len(msizes)), f)
        r0 = 0
        for i, mp_ in enumerate(msizes):
            out[:mp_, i] = v[r0:r0 + mp_]
            r0 += mp_
        return out

    shared = {
        "w1": w1, "w2": w2, "w3": w3, "wh1": wh1, "wh2": wh2, "wh3": wh3,
        "ssum": ssum,
        "b1": col_bias(b1v, L1_M), "b2": col_bias(b2v, L2_M),
        "b3": col_bias(b3v, L3_M), "bh1": col_bias(bh1v, H1_M),
        "bh2": col_bias(bh2v, H1_M), "bh3": bh3v.reshape(A3, 1),
    }

    in_maps = []
    for c in range(NCORES):
        sl = slice(c * R, (c + 1) * R)
        xT = np.ascontiguousarray(x2d[sl].T)                     # [784, R]
        ttc = tt[sl]
        m10 = (np.arange(T)[:, None] == ttc[None, :])
        mask = np.repeat(m10, NCLS, axis=0).astype(f)            # [100, R]
        m = dict(shared)
        m["xT"] = xT
        m["mask"] = mask
        in_maps.append(m)
    return in_maps


def run(inputs, trace=False, **kw):
    if "nc" not in _cache:
        _cache["nc"] = _build_program()
    nc = _cache["nc"]
    inputs = {k: v for k, v in inputs.items() if k != "x_p"}
    in_maps = _prepare_inputs(**inputs)
    res = run_bass_kernel_spmd(nc, in_maps, list(range(NCORES)),
                               trace=trace, **kw)
    outs = [res.results[c]["out"] for c in range(NCORES)]        # [10, R] each
    full = np.concatenate(outs, axis=1)                          # [10, B]
    return np.ascontiguousarray(full.T), res                     # [B, 10]


def kernel(**inputs):
    out, _ = run(inputs, trace=False)
    return out


# revision 4
# speedup vs baseline: 1.0373x; 1.0373x over previous
"""Trainium2 Bass kernel for nn_Net_89094801588965 (moe_routing).

Data-parallel over batch on 8 NeuronCores. Per-core layout puts features on
SBUF partitions and batch on the free dim, so every layer's output is directly
the next layer's moving operand (no transposes on device).

Math (identical to the reference):
  h  = relu(x @ fc1_w + b) -> relu(@fc2_w+b) -> relu(@fc3_w+b)   [B,256]
  p  = relu(x @ priv_w[task_id] + priv_b[task_id])               [B,256]
  xc = [p, h]                                                    [B,512]
  per-task heads t=0..9: a3[t] = (relu(relu(xc@h1w[t]+b)@h2w[t]+b))@h3w[t]+b
  out[b] = a3[tt[b]][b]

Device-side restructuring:
  - fc1 and the private layer share the input x -> fused into one [784,656]
    matmul (cols 0..255 = private, 256..655 = fc1).
  - head layer 1: all tasks packed as [512, 320] (task t at cols 32t..32t+27,
    zero padded) -> [320, N] activations.
  - head layer 2: block-diagonal [320, 320], 128-aligned diagonal blocks ->
    3 matmuls (tasks 0-3, 4-7, 8-9).
  - head layer 3: block-structured [320, 100] -> a3 for all tasks as [100, N].
  - selection: one-hot mask over tasks, expanded to 100 rows on the host,
    applied as (a3 + bias) * mask, then reduced with a constant [100, 10]
    summing matrix on the PE.  (relu/bias of wrong tasks is killed by the
    mask, so masking once at the end is exact.)
All matmuls run as float32r (full PE rate, ~1e-4 relative error).
"""

import sys

sys.path.insert(0, "/opt/trn_rl_repo")

import numpy as np

import concourse.bass as bass
import concourse.mybir as mybir
import concourse.tile as tile
from concourse import bacc
from concourse.bass_utils import run_bass_kernel_spmd

F32 = mybir.dt.float32
F32R = mybir.dt.float32r
RELU = mybir.ActivationFunctionType.Relu

B = 65536
D = 784
HID = 400
LAT = 256
T = 10
NCLS = 10
NCORES = 8
R = B // NCORES          # rows per core
CH = 512                 # batch columns per chunk
NCH = R // CH            # chunks per core

M1 = LAT + HID           # 656 fused L1 output (private | fc1)
HP = 32                  # per-task padded head width
HT = T * HP              # 320
A3 = T * NCLS            # 100

_cache = {}


def _ceil_tiles(n):
    full, rem = divmod(n, 128)
    sizes = [128] * full
    if rem:
        sizes.append(rem)
    return sizes


L1_K = _ceil_tiles(D)            # [128]*6 + [16]
L1_M = _ceil_tiles(M1)           # [128]*5 + [16]
L2_K = _ceil_tiles(HID)          # [128]*3 + [16]
L2_M = _ceil_tiles(HID)
L3_M = _ceil_tiles(LAT)          # [128, 128]
H1_K = _ceil_tiles(2 * LAT)      # [128]*4
H1_M = _ceil_tiles(HT)           # [128, 128, 64]
H3_K = _ceil_tiles(HT)           # [128, 128, 64]


def _build_program():
    nc = bacc.Bacc("TRN2", target_bir_lowering=False, debug=False,
                   num_devices=NCORES)

    xT_d = nc.dram_tensor("xT", [D, R], F32R, kind="ExternalInput")
    mk_d = nc.dram_tensor("mask", [A3, R], F32, kind="ExternalInput")
    w1_d = nc.dram_tensor("w1", [D, M1], F32R, kind="ExternalInput")
    w2_d = nc.dram_tensor("w2", [HID, HID], F32R, kind="ExternalInput")
    w3_d = nc.dram_tensor("w3", [HID, LAT], F32R, kind="ExternalInput")
    wh1_d = nc.dram_tensor("wh1", [2 * LAT, HT], F32R, kind="ExternalInput")
    wh2_d = nc.dram_tensor("wh2", [HT, HT], F32R, kind="ExternalInput")
    wh3_d = nc.dram_tensor("wh3", [HT, A3], F32R, kind="ExternalInput")
    s_d = nc.dram_tensor("ssum", [A3, NCLS], F32R, kind="ExternalInput")
    b1_d = nc.dram_tensor("b1", [128, len(L1_M)], F32, kind="ExternalInput")
    b2_d = nc.dram_tensor("b2", [128, len(L2_M)], F32, kind="ExternalInput")
    b3_d = nc.dram_tensor("b3", [128, len(L3_M)], F32, kind="ExternalInput")
    bh1_d = nc.dram_tensor("bh1", [128, len(H1_M)], F32, kind="ExternalInput")
    bh2_d = nc.dram_tensor("bh2", [128, len(H1_M)], F32, kind="ExternalInput")
    bh3_d = nc.dram_tensor("bh3", [A3, 1], F32, kind="ExternalInput")
    out_d = nc.dram_tensor("out", [NCLS, R], F32, kind="ExternalOutput")

    with tile.TileContext(nc) as tc:
        with (
            tc.tile_pool(name="wp", bufs=1) as wp,
            tc.tile_pool(name="xp", bufs=14) as xp,
            tc.tile_pool(name="mp", bufs=2) as mpool,
            tc.tile_pool(name="ap", bufs=2) as ap,
            tc.tile_pool(name="op", bufs=2) as op,
            tc.tile_pool(name="ps", bufs=8, space="PSUM") as ps,
        ):
            # ---- resident weights -------------------------------------
            def load_w(dram, ksizes, ncols, tag):
                tiles = []
                r0 = 0
                for i, kp in enumerate(ksizes):
                    t = wp.tile([kp, ncols], F32R, tag=f"{tag}{i}")
                    nc.sync.dma_start(t[:], dram[r0:r0 + kp, :])
                    tiles.append(t)
                    r0 += kp
                return tiles

            def load_x_chunk(ci):
                cs = ci * CH
                xk = []
                r0 = 0
                for ki, kp in enumerate(L1_K):
                    t = xp.tile([kp, CH], F32R, tag="x")
                    nc.sync.dma_start(t[:], xT_d[r0:r0 + kp, cs:cs + CH])
                    xk.append(t)
                    r0 += kp
                return xk

            w1 = load_w(w1_d, L1_K, M1, "w1")
            x0k = load_x_chunk(0)
            w2 = load_w(w2_d, L2_K, HID, "w2")
            w3 = load_w(w3_d, L2_K, LAT, "w3")
            wh1 = load_w(wh1_d, H1_K, HT, "wh1")
            wh3 = load_w(wh3_d, H3_K, A3, "wh3")
            # block-diagonal head-2: only the diagonal 128-blocks
            wh2 = []
            r0 = 0
            for i, kp in enumerate(H3_K):
                t = wp.tile([kp, kp], F32R, tag=f"wh2{i}")
                nc.sync.dma_start(t[:], wh2_d[r0:r0 + kp, r0:r0 + kp])
                wh2.append(t)
                r0 += kp
            s_sb = wp.tile([A3, NCLS], F32R, tag="ssum")
            nc.sync.dma_start(s_sb[:], s_d[:])

            def load_b(dram, ncols, tag):
                t = wp.tile([128, ncols], F32, tag=tag)
                nc.sync.dma_start(t[:], dram[:])
                return t

            b1 = load_b(b1_d, len(L1_M), "b1")
            b2 = load_b(b2_d, len(L2_M), "b2")
            b3 = load_b(b3_d, len(L3_M), "b3")
            bh1 = load_b(bh1_d, len(H1_M), "bh1")
            bh2 = load_b(bh2_d, len(H1_M), "bh2")
            bh3 = wp.tile([A3, 1], F32, tag="bh3")
            nc.sync.dma_start(bh3[:], bh3_d[:])

            # ---- per-chunk pipeline -----------------------------------
            def mm_layer(ktiles, wtiles, msizes, psum_tag_prefix, ci):
                """K-accumulated matmuls for one dense layer; returns psum
                tiles (one per m-tile)."""
                psums = []
                c0 = 0
                for mi, mp_ in enumerate(msizes):
                    pt = ps.tile([mp_, CH], F32, tag="ps")
                    nk = len(ktiles)
                    for ki in range(nk):
                        nc.tensor.matmul(
                            pt[:], wtiles[ki][:, c0:c0 + mp_],
                            ktiles[ki][:],
                            start=(ki == 0), stop=(ki == nk - 1),
                        )
                    psums.append(pt)
                    c0 += mp_
                return psums

            def act_relu(psums, bias, msizes, tag, ci, eng="act"):
                outs = []
                for mi, mp_ in enumerate(msizes):
                    t = ap.tile([mp_, CH], F32R, tag=f"{tag}{mi}")
                    if eng == "act":
                        nc.scalar.activation(t[:], psums[mi][:], RELU,
                                             bias=bias[:mp_, mi:mi + 1],
                                             scale=1.0)
                    else:
                        nc.vector.tensor_scalar(
                            t[:], psums[mi][:], bias[:mp_, mi:mi + 1], 0.0,
                            op0=mybir.AluOpType.add, op1=mybir.AluOpType.max)
                    outs.append(t)
                return outs

            deferred_tail = []

            for ci in range(NCH):
                cs = ci * CH
                xk = x0k if ci == 0 else load_x_chunk(ci)
                mk = mpool.tile([A3, CH], F32, tag="mask")
                nc.sync.dma_start(mk[:], mk_d[:, cs:cs + CH])

                # L1 fused (private | fc1)
                ps1 = mm_layer(xk, w1, L1_M, "l1", ci)
                a_l1 = act_relu(ps1, b1, L1_M, "l1o", ci)
                # split: cols 0..255 private (x2 low), 256..655 fc1 hidden
                x2 = [a_l1[0], a_l1[1]]          # p part, [128],[128]
                h1t = [a_l1[2], a_l1[3], a_l1[4], a_l1[5]]  # 128,128,128,16

                # run previous chunk's tail now: its inputs are long since
                # ready, and the PE stays busy with this chunk's L1 above.
                while deferred_tail:
                    deferred_tail.pop(0)()

                # L2
                ps2 = mm_layer(h1t, w2, L2_M, "l2", ci)
                h2t = act_relu(ps2, b2, L2_M, "l2o", ci, eng="dve")
                # L3
                ps3 = mm_layer(h2t, w3, L3_M, "l3", ci)
                x2 += act_relu(ps3, b3, L3_M, "l3o", ci, eng="dve")   # h part

                # head layer 1: [512] -> [320]
                ph1 = mm_layer(x2, wh1, H1_M, "h1", ci)
                a1 = act_relu(ph1, bh1, H1_M, "a1", ci)
                # head layer 2: block diagonal, 3 independent matmuls
                ph2 = []
                for i, kp in enumerate(H3_K):
                    pt = ps.tile([kp, CH], F32, tag="ps")
                    nc.tensor.matmul(pt[:], wh2[i][:], a1[i][:],
                                     start=True, stop=True)
                    ph2.append(pt)
                a2 = act_relu(ph2, bh2, H1_M, "a2", ci, eng="dve")
                # head layer 3: [320] -> [100] (all tasks' logits)
                pt3 = ps.tile([A3, CH], F32, tag="ps")
                for i, kp in enumerate(H3_K):
                    nc.tensor.matmul(pt3[:], wh3[i][:], a2[i][:],
                                     start=(i == 0), stop=(i == len(H3_K) - 1))

                def tail(pt3=pt3, mk=mk, cs=cs):
                    # (a3 + bias) * onehot-mask, then sum tasks via S
                    t2 = ap.tile([A3, CH], F32R, tag="t2")
                    nc.vector.scalar_tensor_tensor(
                        t2[:], pt3[:], bh3[:, 0:1], mk[:],
                        op0=mybir.AluOpType.add, op1=mybir.AluOpType.mult,
                    )
                    po = ps.tile([NCLS, CH], F32, tag="ps")
                    nc.tensor.matmul(po[:], s_sb[:], t2[:],
                                     start=True, stop=True)
                    ot = op.tile([NCLS, CH], F32, tag="o")
                    nc.scalar.copy(ot[:], po[:])
                    nc.sync.dma_start(out_d[:, cs:cs + CH], ot[:])

                deferred_tail.append(tail)

            while deferred_tail:
                deferred_tail.pop(0)()

    nc.compile()
    return nc


def _prepare_inputs(x_s, tt, task_id,
                    fc1_w, fc1_b, fc2_w, fc2_b, fc3_w, fc3_b,
                    priv_w, priv_b, h1_w, h1_b, h2_w, h2_b, h3_w, h3_b):
    f = np.float32
    task_id = int(task_id)

    x2d = np.asarray(x_s, f).reshape(B, D)
    tt = np.asarray(tt).astype(np.int64).reshape(B)

    w1 = np.concatenate([np.asarray(priv_w[task_id], f),
                         np.asarray(fc1_w, f)], axis=1)          # [784, 656]
    b1v = np.concatenate([np.asarray(priv_b[task_id], f),
                          np.asarray(fc1_b, f)])                  # [656]
    w2 = np.ascontiguousarray(np.asarray(fc2_w, f))
    w3 = np.ascontiguousarray(np.asarray(fc3_w, f))
    b2v = np.asarray(fc2_b, f)
    b3v = np.asarray(fc3_b, f)

    wh1 = np.zeros((2 * LAT, HT), f)
    bh1v = np.zeros(HT, f)
    wh2 = np.zeros((HT, HT), f)
    bh2v = np.zeros(HT, f)
    wh3 = np.zeros((HT, A3), f)
    bh3v = np.zeros(A3, f)
    for t in range(T):
        c = HP * t
        wh1[:, c:c + 28] = np.asarray(h1_w[t], f)
        bh1v[c:c + 28] = np.asarray(h1_b[t], f)
        wh2[c:c + 28, c:c + 28] = np.asarray(h2_w[t], f)
        bh2v[c:c + 28] = np.asarray(h2_b[t], f)
        wh3[c:c + 28, NCLS * t:NCLS * (t + 1)] = np.asarray(h3_w[t], f)
        bh3v[NCLS * t:NCLS * (t + 1)] = np.asarray(h3_b[t], f)
    ssum = np.zeros((A3, NCLS), f)
    for t in range(T):
        ssum[NCLS * t:NCLS * (t + 1), :] = np.eye(NCLS, dtype=f)

    def col_bias(v, msizes):
        out = np.zeros((128, len(msizes)), f)
        r0 = 0
        for i, mp_ in enumerate(msizes):
            out[:mp_, i] = v[r0:r0 + mp_]
            r0 += mp_
        return out

    shared = {
        "w1": w1, "w2": w2, "w3": w3, "wh1": wh1, "wh2": wh2, "wh3": wh3,
        "ssum": ssum,
        "b1": col_bias(b1v, L1_M), "b2": col_bias(b2v, L2_M),
        "b3": col_bias(b3v, L3_M), "bh1": col_bias(bh1v, H1_M),
        "bh2": col_bias(bh2v, H1_M), "bh3": bh3v.reshape(A3, 1),
    }

    in_maps = []
    for c in range(NCORES):
        sl = slice(c * R, (c + 1) * R)
        xT = np.ascontiguousarray(x2d[sl].T)                     # [784, R]
        ttc = tt[sl]
        m10 = (np.arange(T)[:, None] == ttc[None, :])
        mask = np.repeat(m10, NCLS, axis=0).astype(f)            # [100, R]
        m = dict(shared)
        m["xT"] = xT
        m["mask"] = mask
        in_maps.append(m)
    return in_maps


def run(inputs, trace=False, **kw):
    if "nc" not in _cache:
        _cache["nc"] = _build_program()
    nc = _cache["nc"]
    inputs = {k: v for k, v in inputs.items() if k != "x_p"}
    in_maps = _prepare_inputs(**inputs)
    res = run_bass_kernel_spmd(nc, in_maps, list(range(NCORES)),
                               trace=trace, **kw)
    outs = [res.results[c]["out"] for c in range(NCORES)]        # [10, R] each
    full = np.concatenate(outs, axis=1)                          # [10, B]
    return np.ascontiguousarray(full.T), res                     # [B, 10]


def kernel(**inputs):
    out, _ = run(inputs, trace=False)
    return out
